# revision 1
# baseline (speedup 1.0000x reference)
"""Trainium2 Bass kernel v2 for nn_CGNN_83605833384509.

Banded-DAG CGNN: gen[:, n] = MLP_n(gen[:, n-4:n] masked, noise[:, n]),
n = 0..63 sequential, B = 262144 batch, data-parallel over 8 cores.

v2 design ("slot-ring, fused phases"): per core, 8 streams of 8 chunks
(W=512 cols). Node-staggered pipeline: at superwave s, node n processes
stream-chunk c = s - n; active window = 8 consecutive nodes. The SBUF
ring X packs, per stream, 4 time slots x 32 rows (8 y rows + 8 noise
rows + 16 zero rows, 32-aligned for engine partition-base rules). All 4
parent lags + noise contract in ONE matmul per (s, stream): lhsT
[128, 80] built per superwave on host; b1 enters via the relu's
per-partition bias. A second matmul per (s, stream) contracts h [97
rows incl a ones row at 96 carrying b2] -> y [8 rows]. Relu and y
evacuation alternate between ACT and DVE. Noise in / gen out move as
1-2 diagonal DMAs per superwave covering all 8 streams at once.

Hardware constraints honored (found via BIR verifier): compute-engine
partition starts must be 32-aligned; matmul psum outs must start at
partition 0 when spanning >32 partitions (so z tiles cycle over 5
psum banks, y outs pack 3-per-bank at bases 0/32/64); matmul psum out
free size is capped at 512 fp32. Issue order keeps <=5 z matmuls in
flight and retires evac(0)/evac(1) early on their engines so the next
superwave's first z-passes are never blocked.
"""

import numpy as np

# ---------------------------------------------------------------- constants
NN = 64          # nodes
KP = 4           # max parents
NH = 10          # hidden width
W = 512          # chunk width (psum bank = 512 fp32)
NS = 8           # streams per core
CS = 8           # chunks per stream
B_SHARD = NS * CS * W          # 32768
N_CORES = 8
B_FULL = B_SHARD * N_CORES
NSW = CS + NN - 1              # 71 superwaves
NSLOT = 4                      # ring slots (32 rows each)
ZROWS = 8 * NH                 # 80 z/h rows per superwave
HONES = 96                     # Hbuf ones row (32-aligned)
NLAG = 2                       # noise DMA lead (superwaves)
OLAG = 1                       # gen DMA lag (superwaves)
NPS = 4                        # psum ring depth for z and for y


def active_nodes(s):
    return range(max(0, s - CS + 1), min(NN - 1, s) + 1)


def yrow(s, n):
    return 32 * (s % NSLOT) + (n % 8)


def nrow(s, n):
    return 32 * (s % NSLOT) + 8 + (n % 8)


# ------------------------------------------------------------- weight packing
def w1_row_for_parent(n, j):
    """W1 slot row holding the weight of parent m = n - j for node n."""
    return KP - j if n >= KP else n - j


def pack_tables(W1, b1, W2, b2):
    """WZ [128, NSW*80] z lhsT; WY [128, NSW*8] y lhsT; B1T [128, NSW]."""
    W1 = np.asarray(W1, np.float32)
    b1 = np.asarray(b1, np.float32)
    W2 = np.asarray(W2, np.float32)
    b2 = np.asarray(b2, np.float32)

    WZ = np.zeros((128, NSW * ZROWS), np.float32)
    WY = np.zeros((128, NSW * 8), np.float32)
    B1T = np.zeros((128, NSW), np.float32)
    for s in range(NSW):
        for n in active_nodes(s):
            c0 = s * ZROWS + NH * (n % 8)
            for j in range(1, KP + 1):
                m = n - j
                if m < 0:
                    continue
                WZ[yrow(s - j, m), c0:c0 + NH] = W1[n, w1_row_for_parent(n, j)]
            WZ[nrow(s, n), c0:c0 + NH] = W1[n, KP]
            B1T[NH * (n % 8):NH * (n % 8) + NH, s] = b1[n]
            cy = s * 8 + (n % 8)
            WY[NH * (n % 8):NH * (n % 8) + NH, cy] = W2[n]
            WY[HONES, cy] = b2[n]
    return WZ, WY, B1T


# ------------------------------------------------------------- DMA job tables
def dma_segments(s):
    """Row segments (row_off, n_lo, k) of the active window, split where
    n % 8 wraps.  row_off is within the 8-row group."""
    n0 = max(0, s - CS + 1)
    n1 = min(NN - 1, s)
    segs = []
    n = n0
    while n <= n1:
        r = n % 8
        k = min(8 - r, n1 - n + 1)
        segs.append((r, n, k))
        n += k
    return segs


# ------------------------------------------------------------- numpy emulator
def emulate_core(noiseT, WZ, WY, B1T):
    """Pure-numpy emulation of the exact kernel schedule."""
    X = np.zeros((128, NS * W), np.float32)
    Hbuf = np.zeros((128, NS * 2 * W), np.float32)
    Hbuf[HONES, :] = 1.0
    G = np.zeros((NN, NS * CS * W), np.float32)

    def noise_in(s):
        if s >= NSW:
            return
        for (r, n_lo, k) in dma_segments(s):
            for kk in range(k):
                n = n_lo + kk
                c = s - n
                for sg in range(NS):
                    X[nrow(s, n), sg * W:(sg + 1) * W] = \
                        noiseT[n, (sg * CS + c) * W:(sg * CS + c + 1) * W]

    def gen_out(s):
        for (r, n_lo, k) in dma_segments(s):
            for kk in range(k):
                n = n_lo + kk
                c = s - n
                for sg in range(NS):
                    G[n, (sg * CS + c) * W:(sg * CS + c + 1) * W] = \
                        X[yrow(s, n), sg * W:(sg + 1) * W]

    for sp in range(NLAG):
        noise_in(sp)
    for s in range(NSW):
        noise_in(s + NLAG)
        for sg in range(NS):
            lhsT = WZ[:, s * ZROWS:(s + 1) * ZROWS]
            rhs = X[:, sg * W:(sg + 1) * W]
            z = lhsT.T @ rhs                                   # [80, W]
            hcol = (sg * 2 + s % 2) * W
            Hbuf[:ZROWS, hcol:hcol + W] = \
                np.maximum(z + B1T[:ZROWS, s:s + 1], 0.0)
            lyT = WY[:HONES + 1, s * 8:(s + 1) * 8]
            y = lyT.T @ Hbuf[:HONES + 1, hcol:hcol + W]        # [8, W]
            X[32 * (s % NSLOT):32 * (s % NSLOT) + 8,
              sg * W:(sg + 1) * W] = y
        if s - OLAG >= 0:
            gen_out(s - OLAG)
    for s in range(max(0, NSW - OLAG), NSW):
        gen_out(s)
    return G


# ------------------------------------------------------------- bass kernel
def build_bass():
    import concourse.bass as bass
    import concourse.bacc as bacc
    import concourse.mybir as mybir
    import concourse.tile as tile

    f32 = mybir.dt.float32
    bf16 = mybir.dt.bfloat16
    RELU = mybir.ActivationFunctionType.Relu
    ADD = mybir.AluOpType.add
    MAX = mybir.AluOpType.max

    nc = bacc.Bacc("TRN2", target_bir_lowering=False, debug=False,
                   enable_asserts=False, num_devices=N_CORES)

    d_noise = nc.dram_tensor("noiseT", [NN, NS * CS * W], bf16,
                             kind="ExternalInput").ap()
    d_wz = nc.dram_tensor("WZ", [128, NSW * ZROWS], bf16,
                          kind="ExternalInput").ap()
    d_wy = nc.dram_tensor("WY", [128, NSW * 8], bf16,
                          kind="ExternalInput").ap()
    d_b1 = nc.dram_tensor("B1T", [128, NSW], f32,
                          kind="ExternalInput").ap()
    d_gen = nc.dram_tensor("gen", [NN, NS * CS * W], bf16,
                           kind="ExternalOutput").ap()

    with tile.TileContext(nc) as tc:
        with tc.tile_pool(name="sb", bufs=1) as sb, \
             tc.tile_pool(name="ps", bufs=1, space="PSUM") as pp:
            X = sb.tile([128, NS * W], bf16)
            Hbuf = sb.tile([128, NS * 2 * W], bf16)
            WZ = sb.tile([128, NSW * ZROWS], bf16)
            WY = sb.tile([128, NSW * 8], bf16)
            B1T = sb.tile([128, NSW], f32)
            # psum: z outs [80, W] must start at partition 0 -> 5 cycling
            # tiles; y outs [8, W] pack 3-per-bank at bases 0/32/64.
            zps = [pp.tile([128, W], f32, name=f"zp{i}") for i in range(5)]
            yts = [pp.tile([128, W], f32, name=f"yt{i}") for i in range(3)]

            def ypsl(sg):
                return yts[sg // 3][32 * (sg % 3):32 * (sg % 3) + 8, :]

            nc.sync.dma_start(WZ[:], d_wz[:])
            nc.sync.dma_start(WY[:], d_wy[:])
            nc.sync.dma_start(B1T[:], d_b1[:])
            nc.vector.memset(X[:], 0.0)
            nc.vector.memset(Hbuf[:], 0.0)
            nc.vector.memset(Hbuf[HONES:HONES + 1, :], 1.0)

            def noise_in(s):
                if s >= NSW:
                    return
                for (r, n_lo, k) in dma_segments(s):
                    off = n_lo * (NS * CS * W) + (s - n_lo) * W
                    src = bass.AP(d_noise.tensor, off,
                                  [[NS * CS * W - W, k], [CS * W, NS],
                                   [1, W]])
                    r0 = 32 * (s % NSLOT) + 8 + r
                    nc.sync.dma_start(X[r0:r0 + k, :], src)

            def gen_out(s):
                for (r, n_lo, k) in dma_segments(s):
                    off = n_lo * (NS * CS * W) + (s - n_lo) * W
                    dst = bass.AP(d_gen.tensor, off,
                                  [[NS * CS * W - W, k], [CS * W, NS],
                                   [1, W]])
                    r0 = 32 * (s % NSLOT) + r
                    nc.sync.dma_start(dst, X[r0:r0 + k, :])

            for sp in range(NLAG):
                noise_in(sp)
            for s in range(NSW):
                noise_in(s + NLAG)
                zrow0 = 32 * (s % NSLOT)
                def z_pass(sg):
                    zp = zps[(s * NS + sg) % 5]
                    nc.tensor.matmul(
                        zp[:ZROWS, :],
                        WZ[:, s * ZROWS:(s + 1) * ZROWS],
                        X[:, sg * W:(sg + 1) * W],
                        start=True, stop=True, skip_group_check=True)

                def relu(sg):
                    zp = zps[(s * NS + sg) % 5]
                    hcol = (sg * 2 + s % 2) * W
                    if sg % 2 == 0:
                        nc.scalar.activation(Hbuf[:ZROWS, hcol:hcol + W],
                                             zp[:ZROWS, :], RELU,
                                             bias=B1T[:ZROWS, s:s + 1])
                    else:
                        nc.vector.tensor_scalar(
                            Hbuf[:ZROWS, hcol:hcol + W], zp[:ZROWS, :],
                            B1T[:ZROWS, s:s + 1], 0.0, ADD, MAX)

                def y_pass(sg):
                    hcol = (sg * 2 + s % 2) * W
                    nc.tensor.matmul(
                        ypsl(sg),
                        WY[:HONES + 1, s * 8:(s + 1) * 8],
                        Hbuf[:HONES + 1, hcol:hcol + W],
                        start=True, stop=True, skip_group_check=True)

                def evac(sg):
                    dst = X[zrow0:zrow0 + 8, sg * W:(sg + 1) * W]
                    if sg % 2 == 0:
                        nc.vector.tensor_scalar_add(dst, ypsl(sg), 0.0)
                    else:
                        nc.scalar.copy(dst, ypsl(sg))

                # interleave so no more than 5 z matmuls are in flight
                # before their relu consumes the psum tile; y/evac pairs
                # issue early for low sg so each engine finishes evac(0|1)
                # quickly and z(s+1, 0..1) is never blocked at the next
                # superwave boundary.
                for sg in range(5):
                    z_pass(sg)
                for sg in range(5, NS):
                    relu(sg - 5)
                    z_pass(sg)
                relu(3)
                y_pass(0)
                evac(0)
                y_pass(1)
                evac(1)
                for sg in range(4, NS):
                    relu(sg)
                    y_pass(sg - 2)
                    evac(sg - 2)
                y_pass(6)
                evac(6)
                y_pass(7)
                evac(7)
                if s - OLAG >= 0:
                    gen_out(s - OLAG)
            for s in range(max(0, NSW - OLAG), NSW):
                gen_out(s)
    return nc


# ------------------------------------------------------------- host kernel
_COMPILED = None
TRACE = False
LAST = None


def kernel(**inputs):
    global _COMPILED, LAST
    noise = np.asarray(inputs["noise"], np.float32)      # [B, 64]
    WZ, WY, B1T = pack_tables(inputs["W1"], inputs["b1"], inputs["W2"],
                              inputs["b2"])

    if _COMPILED is None:
        nc = build_bass()
        nc.compile()
        _COMPILED = nc
    nc = _COMPILED

    import ml_dtypes
    bfnp = ml_dtypes.bfloat16
    noiseT = np.ascontiguousarray(noise.T)               # [64, B]
    wz16, wy16 = WZ.astype(bfnp), WY.astype(bfnp)
    in_maps = []
    for core in range(N_CORES):
        sh = np.ascontiguousarray(
            noiseT[:, core * B_SHARD:(core + 1) * B_SHARD]).astype(bfnp)
        in_maps.append(dict(noiseT=sh, WZ=wz16, WY=wy16, B1T=B1T))

    from concourse.bass_utils import run_bass_kernel_spmd
    res = run_bass_kernel_spmd(nc, in_maps, core_ids=list(range(N_CORES)),
                               trace=TRACE)
    LAST = res
    gen = np.empty((noise.shape[0], NN), np.float32)
    for core in range(N_CORES):
        g = np.asarray(res.results[core]["gen"], np.float32)  # [64, B_SHARD]
        gen[core * B_SHARD:(core + 1) * B_SHARD, :] = g.T
    return gen

